# revision 1
# baseline (speedup 1.0000x reference)
"""Trainium2 Bass kernel for nn_MixDimensionEmbeddingBag (B=16384, F=26, D=64).

Strategy: data-parallel over the batch across 8 NeuronCores, embedding
tables replicated on every core (no collectives needed).  Per core
(2048 samples = 128 partitions x 16 sample-tiles):

  1. Host precomputes per-field row indices (x[:, f] + f*FIELD_DIM) laid
     out partition-major, and ships them as int32 DRAM inputs.
  2. GPSIMD indirect DMA gathers the embedding rows for all three tables
     straight into SBUF (one instruction per table per 4-tile chunk).
  3. DVE tensor_reduce sums over each block's fields.
  4. Blocks 1+2 are projected to 64 dims with one PE matmul per sample
     tile: PE-transpose the summed [128,48] (32+16 concat) to [48,128],
     then matmul against host-stacked [W1.T; W2.T] ([48,64]); bias enters
     via a rank-1 ones x bvec matmul into the same PSUM accumulation.
  5. out = block0_sum + PSUM, DMA'd back to DRAM.
"""

import numpy as np

B = 16384
F = 26
FIELD_DIM = 100000
D = 64
N_CORES = 8
P = 128
T = 16              # sample tiles per core
CT = 4              # sample tiles per gather chunk
NCHUNK = T // CT
BPC = P * T         # samples per core (2048)

_nc_cache = None


def _build(loop_k=None, v0=8 * FIELD_DIM, v1=8 * FIELD_DIM, v2=10 * FIELD_DIM):
    import sys
    try:
        from concourse import bass, bacc, mybir, tile
    except ImportError:
        sys.path.insert(0, "/opt/trn_rl_repo")
        from concourse import bass, bacc, mybir, tile
    from concourse.masks import make_identity

    f32 = mybir.dt.float32
    i32 = mybir.dt.int32
    nc = bacc.Bacc("TRN2", target_bir_lowering=False, debug=False)

    t0 = nc.dram_tensor("t0", [v0, 64], f32, kind="ExternalInput")
    t1 = nc.dram_tensor("t1", [v1, 32], f32, kind="ExternalInput")
    t2 = nc.dram_tensor("t2", [v2, 16], f32, kind="ExternalInput")
    i0 = nc.dram_tensor("i0", [P, T * 8], i32, kind="ExternalInput")
    i1 = nc.dram_tensor("i1", [P, T * 8], i32, kind="ExternalInput")
    i2 = nc.dram_tensor("i2", [P, T * 10], i32, kind="ExternalInput")
    w12 = nc.dram_tensor("w12", [48, 64], f32, kind="ExternalInput")
    bv = nc.dram_tensor("bv", [1, 64], f32, kind="ExternalInput")
    out = nc.dram_tensor("out", [BPC, D], f32, kind="ExternalOutput")

    add = mybir.AluOpType.add
    AX = mybir.AxisListType.X

    with tile.TileContext(nc) as tc:
        with tc.tile_pool(name="const", bufs=1) as cpool, \
             tc.tile_pool(name="g0p", bufs=64) as g0pool, \
             tc.tile_pool(name="g1p", bufs=64) as g1pool, \
             tc.tile_pool(name="g2p", bufs=80) as g2pool, \
             tc.tile_pool(name="tmp", bufs=16) as tpool, \
             tc.tile_pool(name="work", bufs=2) as wpool, \
             tc.tile_pool(name="pst", bufs=2, space="PSUM") as ptpool, \
             tc.tile_pool(name="pso", bufs=2, space="PSUM") as popool:
            ident = cpool.tile([P, P], f32)
            make_identity(nc, ident[:])
            ones = cpool.tile([1, P], f32)
            nc.gpsimd.memset(ones[:], 1.0)
            i0_sb = cpool.tile([P, T * 8], i32)
            i1_sb = cpool.tile([P, T * 8], i32)
            i2_sb = cpool.tile([P, T * 10], i32)
            w12_sb = cpool.tile([48, 64], f32)
            bv_sb = cpool.tile([1, 64], f32)
            nc.sync.dma_start(out=i0_sb[:], in_=i0[:])
            nc.sync.dma_start(out=i1_sb[:], in_=i1[:])
            nc.sync.dma_start(out=i2_sb[:], in_=i2[:])
            nc.sync.dma_start(out=w12_sb[:], in_=w12[:])
            nc.sync.dma_start(out=bv_sb[:], in_=bv[:])

            out_v = out[:].rearrange("(p t) d -> p t d", t=T)

            def gather_one(table, idx_sb, j, width, pool):
                # HW indirect DMA consumes ONE offset per partition and
                # streams the dest free extent from that row: one
                # instruction gathers exactly 128 rows.  Each gather gets
                # its OWN pool tile — slice-writes into a shared tile make
                # Tile serialize the DMAs at completion latency (~2.8us
                # each, measured), 3.3x slower than the SWDGE issue rate.
                gt = pool.tile([P, width], f32)
                nc.gpsimd.indirect_dma_start(
                    out=gt[:], out_offset=None, in_=table[:],
                    in_offset=bass.IndirectOffsetOnAxis(
                        ap=idx_sb[:, j:j + 1], axis=0))
                return gt

            def tree_sum(tiles, out_slice, width):
                cur = list(tiles)
                while len(cur) > 2:
                    nxt = []
                    for a in range(0, len(cur) - 1, 2):
                        ts = tpool.tile([P, width], f32)
                        nc.vector.tensor_add(
                            out=ts[:], in0=cur[a][:], in1=cur[a + 1][:])
                        nxt.append(ts)
                    if len(cur) % 2:
                        nxt.append(cur[-1])
                    cur = nxt
                if len(cur) == 2:
                    nc.vector.tensor_add(
                        out=out_slice, in0=cur[0][:], in1=cur[1][:])
                else:
                    nc.vector.tensor_copy(out=out_slice, in_=cur[0][:])

            def chunk_body():
                for c in range(NCHUNK):
                    gt0 = [gather_one(t0, i0_sb, c * CT * 8 + s, 64, g0pool)
                           for s in range(CT * 8)]
                    gt1 = [gather_one(t1, i1_sb, c * CT * 8 + s, 32, g1pool)
                           for s in range(CT * 8)]
                    gt2 = [gather_one(t2, i2_sb, c * CT * 10 + s, 16, g2pool)
                           for s in range(CT * 10)]

                    b0s = wpool.tile([P, CT * 64], f32)
                    e12 = wpool.tile([P, CT * 48], f32)
                    for t in range(CT):
                        tree_sum(gt0[t * 8:(t + 1) * 8],
                                 b0s[:, t * 64:(t + 1) * 64], 64)
                        tree_sum(gt1[t * 8:(t + 1) * 8],
                                 e12[:, t * 48:t * 48 + 32], 32)
                        tree_sum(gt2[t * 10:(t + 1) * 10],
                                 e12[:, t * 48 + 32:(t + 1) * 48], 16)

                    osb = wpool.tile([P, CT * 64], f32)
                    for j in range(CT):
                        pt = ptpool.tile([P, P], f32)
                        lt = wpool.tile([48, P], f32)
                        po = popool.tile([P, 64], f32)
                        nc.tensor.transpose(
                            out=pt[:48, :], in_=e12[:, j * 48:(j + 1) * 48],
                            identity=ident[:])
                        nc.vector.tensor_copy(out=lt[:], in_=pt[:48, :])
                        nc.tensor.matmul(
                            po[:], lhsT=lt[:], rhs=w12_sb[:], start=True, stop=False)
                        nc.tensor.matmul(
                            po[:], lhsT=ones[:], rhs=bv_sb[:], start=False, stop=True)
                        nc.vector.tensor_add(
                            out=osb[:, j * 64:(j + 1) * 64],
                            in0=b0s[:, j * 64:(j + 1) * 64], in1=po[:])
                    nc.sync.dma_start(
                        out=out_v[:, c * CT:(c + 1) * CT, :],
                        in_=osb[:].rearrange("p (t d) -> p t d", t=CT))

            if loop_k:
                with tc.For_i(0, loop_k, 1) as iv:
                    chunk_body()
            else:
                chunk_body()

    nc.compile()
    return nc


def _make_in_maps(x, t0, t1, t2, W1, b1, W2, b2):
    x = np.ascontiguousarray(np.asarray(x)).astype(np.int32, copy=False)
    t0 = np.ascontiguousarray(np.asarray(t0, dtype=np.float32))
    t1 = np.ascontiguousarray(np.asarray(t1, dtype=np.float32))
    t2 = np.ascontiguousarray(np.asarray(t2, dtype=np.float32))
    W1 = np.asarray(W1, dtype=np.float32)
    W2 = np.asarray(W2, dtype=np.float32)
    b1 = np.asarray(b1, dtype=np.float32)
    b2 = np.asarray(b2, dtype=np.float32)
    w12 = np.ascontiguousarray(
        np.concatenate([W1.T, W2.T], axis=0).astype(np.float32))  # [48, 64]
    bv = (8.0 * b1 + 10.0 * b2).astype(np.float32).reshape(1, D)

    off0 = np.arange(8, dtype=np.int32) * FIELD_DIM
    off2 = np.arange(10, dtype=np.int32) * FIELD_DIM
    in_maps = []
    for c in range(N_CORES):
        xs = x[c * BPC:(c + 1) * BPC].reshape(P, T, F)
        ia = np.ascontiguousarray(
            (xs[:, :, 0:8] + off0).reshape(P, T * 8).astype(np.int32))
        ib = np.ascontiguousarray(
            (xs[:, :, 8:16] + off0).reshape(P, T * 8).astype(np.int32))
        ic = np.ascontiguousarray(
            (xs[:, :, 16:26] + off2).reshape(P, T * 10).astype(np.int32))
        in_maps.append({
            "t0": t0, "t1": t1, "t2": t2,
            "i0": ia, "i1": ib, "i2": ic,
            "w12": w12, "bv": bv,
        })
    return in_maps


def kernel(x, t0, t1, t2, W1, b1, W2, b2, _trace=False):
    global _nc_cache
    import sys
    try:
        from concourse.bass_utils import run_bass_kernel_spmd
    except ImportError:
        sys.path.insert(0, "/opt/trn_rl_repo")
        from concourse.bass_utils import run_bass_kernel_spmd

    in_maps = _make_in_maps(x, t0, t1, t2, W1, b1, W2, b2)
    if _nc_cache is None:
        _nc_cache = _build()
    res = run_bass_kernel_spmd(
        _nc_cache, in_maps, list(range(N_CORES)), trace=_trace)
    out = np.concatenate(
        [np.asarray(res.results[c]["out"]) for c in range(N_CORES)], axis=0)
    if _trace:
        return out, res
    return out

